# revision 2
# baseline (speedup 1.0000x reference)
"""Segment-mean reduction (grouped mean over sorted segment ids) on 8 trn2 cores.

Strategy (data-parallel over batch): each core handles one batch row.
out[g, :] = mean over rows s of feats with segment_ids[s] == g.

Because segment_ids are sorted per row, each 128-row tile of feats touches a
small contiguous range of group-chunks. On the host (at trace time) we compute,
per tile, the window of 128-group chunks it can touch (union across cores so
the SPMD program is identical), and bake a static matmul schedule:

    psum[chunk] += M_tile.T @ feats_tile

where M_tile[s, g_local] = (g_local == seg[s] - base) * (1/count[seg[s]]) is a
one-hot-times-reciprocal weight built on-device with a single fused
tensor_scalar op (iota is_equal sl) * rc. PSUM accumulates each 128-group chunk
across its contributing tiles; a copy + DMA stores the means.

Per-core HBM traffic ~= feats in (8 MB) + out (1 MB) + tiny aux => memory-bound.
"""

import os
import numpy as np

import concourse.bass as bass
import concourse.bacc as bacc
import concourse.mybir as mybir
import concourse.tile as tile
from concourse.bass_utils import run_bass_kernel_spmd

F32 = mybir.dt.float32
P = 128  # partitions


def _host_schedule(seg_all: np.ndarray, G: int):
    """seg_all: [R, S] sorted int32. Returns per-tile union chunk windows and
    per-chunk first/last tile, plus per-core aux arrays (sl, rc)."""
    R, S = seg_all.shape
    T = S // P
    CH = G // P

    c_lo = np.empty((R, T), np.int64)
    c_hi = np.empty((R, T), np.int64)
    for r in range(R):
        seg = seg_all[r]
        c_lo[r] = seg[0::P][:T] // P if False else seg[np.arange(T) * P] // P
        c_hi[r] = seg[np.arange(T) * P + (P - 1)] // P
    C_lo = c_lo.min(axis=0)  # [T]
    C_hi = c_hi.max(axis=0)  # [T]
    W = (C_hi - C_lo + 1).astype(np.int64)  # [T]
    Wmax = int(W.max())

    # chunk -> contributing tiles (static, union across cores)
    first = np.full(CH, -1, np.int64)
    last = np.full(CH, -1, np.int64)
    for t in range(T):
        for c in range(int(C_lo[t]), int(C_hi[t]) + 1):
            if first[c] < 0:
                first[c] = t
            last[c] = t

    # per-core aux: sl = seg - 128*C_lo[tile], rc = 1/count[seg]
    aux_sl = np.empty((R, P, T), np.float32)
    aux_rc = np.empty((R, P, T), np.float32)
    for r in range(R):
        seg = seg_all[r]
        counts = np.bincount(seg, minlength=G).astype(np.float32)
        recip = 1.0 / np.maximum(counts, 1.0)
        sl = seg.astype(np.int64) - (C_lo[np.arange(S) // P] * P)
        aux_sl[r] = sl.astype(np.float32).reshape(T, P).T
        aux_rc[r] = recip[seg].reshape(T, P).T

    return dict(C_lo=C_lo, W=W, Wmax=Wmax, first=first, last=last,
                aux_sl=aux_sl, aux_rc=aux_rc, T=T, CH=CH)


def _build_program(S: int, H: int, G: int, sched, grp: int = 8):
    """Build the SPMD bass program (identical across cores)."""
    T, CH, Wmax = sched["T"], sched["CH"], sched["Wmax"]
    C_lo, W = sched["C_lo"], sched["W"]
    first, last = sched["first"], sched["last"]
    WIN = Wmax * P

    nc = bacc.Bacc("TRN2", target_bir_lowering=False, debug=False, num_devices=8)
    feats_d = nc.dram_tensor("feats", [S, H], F32, kind="ExternalInput")
    sl_d = nc.dram_tensor("aux_sl", [P, T], F32, kind="ExternalInput")
    rc_d = nc.dram_tensor("aux_rc", [P, T], F32, kind="ExternalInput")
    iota_d = nc.dram_tensor("iota", [P, WIN], F32, kind="ExternalInput")
    out_d = nc.dram_tensor("out", [G, H], F32, kind="ExternalOutput")

    with tile.TileContext(nc) as tc:
        with (
            tc.tile_pool(name="const", bufs=1) as constp,
            tc.tile_pool(name="feats", bufs=3) as fpool,
            tc.tile_pool(name="mt", bufs=4) as mtpool,
            tc.tile_pool(name="outp", bufs=2) as opool,
            tc.tile_pool(name="psum", bufs=1, space="PSUM") as pp,
        ):
            iota_t = constp.tile([P, WIN], F32, tag="iota")
            nc.sync.dma_start(iota_t[:], iota_d.ap())
            sl_t = constp.tile([P, T], F32, tag="sl")
            nc.sync.dma_start(sl_t[:], sl_d.ap())
            rc_t = constp.tile([P, T], F32, tag="rc")
            nc.sync.dma_start(rc_t[:], rc_d.ap())

            psum_tiles = [
                pp.tile([P, H], F32, tag=f"ps{c}", name=f"ps{c}") for c in range(CH)
            ]

            # feats rows (a p) -> partition p, free (a, h)
            feats_v = feats_d.ap().rearrange("(a p) h -> p a h", p=P)

            for g0 in range(T // grp):
                ft = fpool.tile([P, grp, H], F32, tag="ft")
                nc.sync.dma_start(ft[:], feats_v[:, g0 * grp:(g0 + 1) * grp, :])
                for tt in range(grp):
                    t = g0 * grp + tt
                    w = int(W[t])
                    mt = mtpool.tile([P, WIN], F32, tag="mt")
                    # M[s, j] = (iota[j] == sl[s]) * rc[s]
                    nc.vector.tensor_scalar(
                        mt[:, : w * P],
                        iota_t[:, : w * P],
                        sl_t[:, t:t + 1],
                        rc_t[:, t:t + 1],
                        mybir.AluOpType.is_equal,
                        mybir.AluOpType.mult,
                    )
                    for j in range(w):
                        c = int(C_lo[t]) + j
                        nc.tensor.matmul(
                            psum_tiles[c][:],
                            mt[:, j * P:(j + 1) * P],
                            ft[:, tt, :],
                            start=(t == first[c]),
                            stop=(t == last[c]),
                        )

            for c in range(CH):
                ot = opool.tile([P, H], F32, tag="ot")
                if first[c] >= 0:
                    nc.scalar.copy(ot[:], psum_tiles[c][:])
                else:
                    nc.vector.memset(ot[:], 0.0)
                nc.sync.dma_start(out_d.ap()[c * P:(c + 1) * P, :], ot[:])

    nc.compile()
    return nc


def kernel(feats, segment_ids, num_groups, _trace=False):
    feats = np.ascontiguousarray(np.asarray(feats, dtype=np.float32))
    seg_all = np.ascontiguousarray(np.asarray(segment_ids, dtype=np.int32))
    G = int(num_groups)
    B, S, H = feats.shape
    assert seg_all.shape == (B, S) and B == 8 and G % P == 0 and S % P == 0

    sched = _host_schedule(seg_all, G)
    nc = _build_program(S, H, G, sched)

    WIN = sched["Wmax"] * P
    iota_arr = np.broadcast_to(
        np.arange(WIN, dtype=np.float32)[None, :], (P, WIN)
    ).copy()

    in_maps = [
        {
            "feats": feats[r],
            "aux_sl": sched["aux_sl"][r],
            "aux_rc": sched["aux_rc"][r],
            "iota": iota_arr,
        }
        for r in range(B)
    ]
    res = run_bass_kernel_spmd(nc, in_maps, list(range(B)), trace=_trace)
    out = np.stack([res.results[r]["out"] for r in range(B)])
    if _trace:
        return out, res
    return out
